# revision 7
# baseline (speedup 1.0000x reference)
"""Multi-head attention with q/v LoRA on 8 trn2 NeuronCores.

Reference computation (B=2, N=2048, C=1024, H=16, HD=64, R=16):
    qkv = x @ w_qkv + b_qkv                -> split per-head q, k, v
    q  += ((q @ a_q) @ b_q) * 2.0          (per head; same for v)
    out = softmax(q k^T / 8) v             (full N x N scores)
    y   = out @ w_proj + b_proj

Sharding: tensor-parallel over heads -- each of the 8 cores owns 2 heads
(128 of the 1024 qkv columns / proj rows) for both batches.  Each core:
  1. transposes x on the PE (fp32) and computes its qkv^T shard in fp32r,
  2. applies LoRA via block-diagonal [128,32]/[32,128] matrices,
  3. per (batch, head): scores S^T = k^T' q^T chunks -> exp on ACT ->
     P @ [v | 1] accumulated in PSUM (ones column yields softmax sums),
     then normalizes with a PE ones-broadcast of the reciprocal sums,
  4. computes its partial y^T = w_proj_shard^T-contraction and
     ReduceScatters the [1024, 2048] per-batch partials so every core
     ends with a distinct 128-row shard of the summed y^T.
The host stitches the 8 shards and transposes back to [B, N, C].
"""

import sys

sys.path.insert(0, "/opt/trn_rl_repo")
sys.path.insert(0, "/root/.axon_site")

import numpy as np

import concourse.bass as bass
import concourse.mybir as mybir
import concourse.tile as tile
from concourse.bass_utils import run_bass_kernel_spmd

f32 = mybir.dt.float32
f32r = mybir.dt.float32r
AF = mybir.ActivationFunctionType

B, N, C = 2, 2048, 1024
H, HD, R = 16, 64, 16
LORA_SCALE = 32.0 / R
ATTN_SCALE = HD ** -0.5
NCORES = 8
HPC = H // NCORES          # heads per core = 2
PC = HPC * HD              # partition columns per core = 128
ROWS = B * N               # 4096 tokens
NRC = ROWS // 512          # 512-token row chunks


def _legalize_waits(nc, max_waits=1):
    """This walrus build accepts at most one sync-wait per instruction;
    Tile attaches several.  Move surplus waits onto same-engine NoOps
    inserted immediately before the instruction (identical semantics)."""
    counter = 0
    for fn in nc.m.functions:
        for bb in fn.blocks:
            insts = bb.instructions
            out = []
            changed = False
            for inst in insts:
                si = inst.sync_info
                if si is not None and si.on_wait and len(si.on_wait) > max_waits:
                    waits = list(si.on_wait)
                    for w in waits[:-max_waits]:
                        counter += 1
                        nop = mybir.InstNoOp(
                            name=f"I-wfix-{counter}",
                            engine=inst.engine,
                            sync_info=mybir.SyncInfo(on_wait=[w], on_update=[]),
                        )
                        nc.register_instruction(nop)
                        out.append(nop)
                    si.on_wait.clear()
                    si.on_wait.extend(waits[-max_waits:])
                    changed = True
                out.append(inst)
            if changed:
                insts[:] = out


def build_nc():
    nc = bass.Bass(num_devices=NCORES)

    x_d = nc.dram_tensor("x", [ROWS, C], f32, kind="ExternalInput")
    wq_d = nc.dram_tensor("wq", [128, 1024], f32, kind="ExternalInput")
    wk_d = nc.dram_tensor("wk", [128, 1024], f32, kind="ExternalInput")
    wv_d = nc.dram_tensor("wv", [128, 1024], f32, kind="ExternalInput")
    bq_d = nc.dram_tensor("bq", [128, 1], f32, kind="ExternalInput")
    bk_d = nc.dram_tensor("bk", [128, 1], f32, kind="ExternalInput")
    bv_d = nc.dram_tensor("bv", [128, 1], f32, kind="ExternalInput")
    a2q_d = nc.dram_tensor("a2q", [128, 2 * R], f32, kind="ExternalInput")
    b2q_d = nc.dram_tensor("b2q", [2 * R, 128], f32, kind="ExternalInput")
    a2v_d = nc.dram_tensor("a2v", [128, 2 * R], f32, kind="ExternalInput")
    b2v_d = nc.dram_tensor("b2v", [2 * R, 128], f32, kind="ExternalInput")
    wp_d = nc.dram_tensor("wp", [128, 1024], f32, kind="ExternalInput")
    bp_d = nc.dram_tensor("bp", [128, 8], f32, kind="ExternalInput")
    eye128_d = nc.dram_tensor("eye128", [128, 128], f32, kind="ExternalInput")
    eye64x2_d = nc.dram_tensor("eye64x2", [128, 64], f32, kind="ExternalInput")
    out_d = nc.dram_tensor("out", [B, 128, N], f32, kind="ExternalOutput")

    with nc.allow_low_precision(reason="fp32r tensors feed fp32r matmuls; PSUM accumulation stays fp32"), tile.TileContext(nc) as tc:
        with (
            tc.tile_pool(name="persist", bufs=1) as persist,
            tc.tile_pool(name="const", bufs=1) as const,
        ):
            # ---- persistent SBUF tensors ---------------------------------
            qT = persist.tile([128, ROWS], f32r, tag="qT", name="qT")
            kT = persist.tile([128, ROWS], f32r, tag="kT", name="kT")
            vT = persist.tile([128, ROWS], f32r, tag="vT", name="vT")
            attnT = persist.tile([128, ROWS], f32r, tag="attnT", name="attnT")
            # fp32 staging + on-device rounding to fp32r for matmul operands
            def rounded(name, dram, shape):
                stg = const.tile(list(shape), f32, tag="stg", name=f"{name}_stg")
                nc.sync.dma_start(out=stg[:], in_=dram[:])
                t = const.tile(list(shape), f32r, tag=name, name=name)
                nc.vector.tensor_copy(t[:], stg[:])
                return t

            w_t = [
                rounded("wq_t", wq_d, (128, 1024)),
                rounded("wk_t", wk_d, (128, 1024)),
                rounded("wv_t", wv_d, (128, 1024)),
            ]
            wp_t = rounded("wp_t", wp_d, (128, 1024))
            a2q_t = rounded("a2q_t", a2q_d, (128, 2 * R))
            b2q_t = rounded("b2q_t", b2q_d, (2 * R, 128))
            a2v_t = rounded("a2v_t", a2v_d, (128, 2 * R))
            b2v_t = rounded("b2v_t", b2v_d, (2 * R, 128))

            eye128 = const.tile([128, 128], f32, tag="eye128", name="eye128")
            nc.sync.dma_start(out=eye128[:], in_=eye128_d[:])
            eye64x2_s = const.tile([128, 64], f32, tag="eye64s", name="eye64s")
            nc.sync.dma_start(out=eye64x2_s[:], in_=eye64x2_d[:])
            eye64x2 = const.tile([128, 64], f32r, tag="eye64", name="eye64")
            nc.vector.tensor_copy(eye64x2[:], eye64x2_s[:])

            ones_s = const.tile([1, 64], f32, tag="ones_s", name="ones_s")
            nc.gpsimd.memset(ones_s[:], 1.0)
            ones_row = const.tile([1, 64], f32r, tag="ones_r", name="ones_r")
            nc.vector.tensor_copy(ones_row[:], ones_s[:])
            ones_vaug = const.tile([128, 16 * 65], f32, tag="ones_v", name="ones_v")
            nc.gpsimd.memset(ones_vaug[:], 1.0)

            bias_t = []
            for nm, d in (("bq", bq_d), ("bk", bk_d), ("bv", bv_d)):
                bt = const.tile([128, 1], f32, tag=nm, name=f"{nm}_t")
                nc.sync.dma_start(out=bt[:], in_=d[:])
                bias_t.append(bt)
            bp_t = const.tile([128, 8], f32, tag="bp", name="bp_t")
            nc.sync.dma_start(out=bp_t[:], in_=bp_d[:])

            with (
                tc.tile_pool(name="dram", bufs=1, space="DRAM") as dram,
                tc.tile_pool(name="xrow", bufs=5) as xrow_p,
                tc.tile_pool(name="xT", bufs=2) as xT_p,
                tc.tile_pool(name="work", bufs=2) as work_p,
                tc.tile_pool(name="ystage", bufs=4) as ystage_p,
            ):
                # ---- phase A: x^T + qkv^T --------------------------------
                with tc.tile_pool(name="psA", bufs=1, space="PSUM") as psA:
                    qkvT = (qT, kT, vT)
                    for rc in range(NRC):
                        xrows = []
                        for j in range(4):
                            xr = xrow_p.tile([128, C], f32, tag="xr", name=f"xr{rc}_{j}")
                            nc.sync.dma_start(
                                out=xr[:], in_=x_d[rc * 512 + j * 128 : rc * 512 + (j + 1) * 128, :]
                            )
                            xrows.append(xr)
                        xT_t = xT_p.tile([128, 4096], f32r, tag="xT", name=f"xT{rc}")
                        for ci in range(8):
                            tp = psA.tile([128, 512], f32, tag="tr", bufs=3, name=f"tp{rc}_{ci}")
                            for j in range(4):
                                nc.tensor.transpose(
                                    tp[:, j * 128 : (j + 1) * 128],
                                    xrows[j][:, ci * 128 : (ci + 1) * 128],
                                    eye128[:],
                                )
                            nc.vector.tensor_copy(xT_t[:, ci * 512 : (ci + 1) * 512], tp[:])
                        for m in range(3):
                            acc = psA.tile([128, 512], f32, tag="acc", bufs=3, name=f"acc{rc}_{m}")
                            for ci in range(8):
                                nc.tensor.matmul(
                                    acc[:],
                                    w_t[m][:, ci * 128 : (ci + 1) * 128],
                                    xT_t[:, ci * 512 : (ci + 1) * 512],
                                    start=(ci == 0),
                                    stop=(ci == 7),
                                )
                            nc.vector.tensor_scalar_add(
                                qkvT[m][:, rc * 512 : (rc + 1) * 512], acc[:], bias_t[m][:]
                            )

                    # ---- LoRA on q and v (block-diagonal, both heads) ----
                    for dstT, a2, b2 in ((qT, a2q_t, b2q_t), (vT, a2v_t, b2v_t)):
                        for ch in range(NRC):
                            sl = slice(ch * 512, (ch + 1) * 512)
                            t_ps = psA.tile([2 * R, 512], f32, tag="acc", bufs=3, name=f"tps{ch}")
                            nc.tensor.matmul(t_ps[:], a2[:], dstT[:, sl], start=True, stop=True)
                            t_sb = work_p.tile([2 * R, 512], f32r, tag="lt", name=f"tsb{ch}")
                            nc.vector.tensor_copy(t_sb[:], t_ps[:])
                            d_ps = psA.tile([128, 512], f32, tag="acc", bufs=3, name=f"dps{ch}")
                            nc.tensor.matmul(d_ps[:], b2[:], t_sb[:], start=True, stop=True)
                            nc.vector.tensor_add(dstT[:, sl], dstT[:, sl], d_ps[:])

                # ---- phase B: attention + proj + reduce-scatter ----------
                with tc.tile_pool(name="psB", bufs=1, space="PSUM") as psB:
                    for b in range(B):
                        boff = b * N
                        for hl in range(HPC):
                            hs = slice(hl * HD, (hl + 1) * HD)
                            v_aug = work_p.tile([128, 16 * 65], f32r, tag="vaug", name=f"va{b}{hl}")
                            nc.vector.tensor_copy(v_aug[:], ones_vaug[:])
                            o_ps = psB.tile([65, N], f32, tag="o", bufs=1, name=f"o{b}{hl}")
                            for kt in range(16):
                                ko = boff + kt * 128
                                vtr = psB.tile([128, 64], f32r, tag="s", bufs=2, name=f"vt{kt}")
                                nc.tensor.transpose(
                                    vtr[:], vT[hs, ko : ko + 128], eye64x2[hs, :]
                                )
                                nc.vector.tensor_copy(
                                    v_aug[:, kt * 65 : kt * 65 + 64], vtr[:]
                                )
                                p_sb = work_p.tile([128, N], f32r, tag="p", name=f"p{kt}")
                                for qh in range(2):
                                    s_ps = psB.tile(
                                        [128, 1024], f32, tag="s", bufs=2, name=f"s{kt}_{qh}"
                                    )
                                    for qc in range(2):
                                        qo = boff + qh * 1024 + qc * 512
                                        nc.tensor.matmul(
                                            s_ps[:, qc * 512 : (qc + 1) * 512],
                                            kT[hs, ko : ko + 128],
                                            qT[hs, qo : qo + 512],
                                            start=True,
                                            stop=True,
                                        )
                                    nc.scalar.activation(
                                        p_sb[:, qh * 1024 : (qh + 1) * 1024],
                                        s_ps[:],
                                        AF.Exp,
                                        scale=ATTN_SCALE,
                                    )
                                for qc4 in range(4):
                                    nc.tensor.matmul(
                                        o_ps[:, qc4 * 512 : (qc4 + 1) * 512],
                                        v_aug[:, kt * 65 : kt * 65 + 65],
                                        p_sb[:, qc4 * 512 : (qc4 + 1) * 512],
                                        start=(kt == 0),
                                        stop=(kt == 15),
                                    )
                            # normalize: rows 0..63 are O^T, row 64 the sums
                            r_sb = work_p.tile([1, N], f32r, tag="r", name=f"r{b}{hl}")
                            nc.vector.reciprocal(r_sb[:], o_ps[64:65, :])
                            for qh in range(2):
                                bc_ps = psB.tile(
                                    [128, 1024], f32, tag="s", bufs=2, name=f"bc{qh}"
                                )
                                for qc in range(2):
                                    nc.tensor.matmul(
                                        bc_ps[0:64, qc * 512 : (qc + 1) * 512],
                                        ones_row[:],
                                        r_sb[:, qh * 1024 + qc * 512 : qh * 1024 + (qc + 1) * 512],
                                        start=True,
                                        stop=True,
                                    )
                                bc_sb = work_p.tile(
                                    [64, 1024], f32, tag="bcs", name=f"bcs{qh}"
                                )
                                nc.vector.tensor_copy(bc_sb[:], bc_ps[0:64, :])
                                nc.vector.tensor_mul(
                                    attnT[hs, boff + qh * 1024 : boff + (qh + 1) * 1024],
                                    o_ps[0:64, qh * 1024 : (qh + 1) * 1024],
                                    bc_sb[:],
                                )

                        # ---- proj partial for this batch ------------------
                        yp = dram.tile([C, N], f32, tag=f"yp{b}", name=f"yp{b}")
                        for mt in range(8):
                            for rc2 in range(4):
                                y_ps = psB.tile(
                                    [128, 512], f32, tag="s", bufs=2, name=f"y{mt}_{rc2}"
                                )
                                nc.tensor.matmul(
                                    y_ps[:],
                                    wp_t[:, mt * 128 : (mt + 1) * 128],
                                    attnT[:, boff + rc2 * 512 : boff + (rc2 + 1) * 512],
                                    start=True,
                                    stop=True,
                                )
                                yst = ystage_p.tile([128, 512], f32, tag="yst", name=f"ys{mt}{rc2}")
                                nc.vector.tensor_scalar_add(
                                    yst[:], y_ps[:], bp_t[:, mt : mt + 1]
                                )
                                nc.sync.dma_start(
                                    out=yp[mt * 128 : (mt + 1) * 128, rc2 * 512 : (rc2 + 1) * 512],
                                    in_=yst[:],
                                )
                        yr = dram.tile([128, N], f32, tag=f"yr{b}", name=f"yr{b}")
                        nc.gpsimd.collective_compute(
                            "ReduceScatter",
                            mybir.AluOpType.add,
                            replica_groups=[list(range(NCORES))],
                            ins=[yp[:].opt()],
                            outs=[yr[:].opt()],
                        )
                        nc.sync.dma_start(out=out_d[b], in_=yr[:])

    _legalize_waits(nc)
    return nc


_NC_CACHE = None


def _get_nc():
    global _NC_CACHE
    if _NC_CACHE is None:
        _NC_CACHE = build_nc()
    return _NC_CACHE


def _make_in_maps(inputs):
    x = np.ascontiguousarray(np.asarray(inputs["x"], dtype=np.float32)).reshape(ROWS, C)
    w_qkv = np.asarray(inputs["w_qkv"], dtype=np.float32)
    b_qkv = np.asarray(inputs["b_qkv"], dtype=np.float32)
    a_q = np.asarray(inputs["a_q"], dtype=np.float32)
    b_q = np.asarray(inputs["b_q"], dtype=np.float32)
    a_v = np.asarray(inputs["a_v"], dtype=np.float32)
    b_v = np.asarray(inputs["b_v"], dtype=np.float32)
    w_proj = np.asarray(inputs["w_proj"], dtype=np.float32)
    b_proj = np.asarray(inputs["b_proj"], dtype=np.float32)

    def blkdiag(m):
        z = np.zeros((2 * m.shape[0], 2 * m.shape[1]), dtype=np.float32)
        z[: m.shape[0], : m.shape[1]] = m
        z[m.shape[0] :, m.shape[1] :] = m
        return z

    a2q = blkdiag(a_q)                        # [128, 32]
    b2q = blkdiag(b_q) * LORA_SCALE           # [32, 128]
    a2v = blkdiag(a_v)
    b2v = blkdiag(b_v) * LORA_SCALE
    eye128 = np.eye(128, dtype=np.float32)
    eye64x2 = np.vstack([np.eye(64, dtype=np.float32)] * 2)

    def warr(w):                              # [1024, 128] -> [128, 8*128]
        return np.ascontiguousarray(
            w.reshape(8, 128, 128).transpose(1, 0, 2).reshape(128, 1024)
        )

    in_maps = []
    for c in range(NCORES):
        cs = slice(c * PC, (c + 1) * PC)
        bp = b_proj if c == 0 else np.zeros_like(b_proj)
        in_maps.append(
            {
                "x": x,
                "wq": warr(w_qkv[:, 0 * C + c * PC : 0 * C + (c + 1) * PC]),
                "wk": warr(w_qkv[:, 1 * C + c * PC : 1 * C + (c + 1) * PC]),
                "wv": warr(w_qkv[:, 2 * C + c * PC : 2 * C + (c + 1) * PC]),
                "bq": np.ascontiguousarray(b_qkv[0 * C + c * PC : 0 * C + (c + 1) * PC].reshape(128, 1)),
                "bk": np.ascontiguousarray(b_qkv[1 * C + c * PC : 1 * C + (c + 1) * PC].reshape(128, 1)),
                "bv": np.ascontiguousarray(b_qkv[2 * C + c * PC : 2 * C + (c + 1) * PC].reshape(128, 1)),
                "a2q": a2q,
                "b2q": b2q,
                "a2v": a2v,
                "b2v": b2v,
                "wp": np.ascontiguousarray(w_proj[cs, :]),
                "bp": np.ascontiguousarray(bp.reshape(8, 128).T),
                "eye128": eye128,
                "eye64x2": eye64x2,
            }
        )
    return in_maps


def run_sharded(inputs, trace=False, **kw):
    nc = _get_nc()
    in_maps = _make_in_maps(inputs)
    res = run_bass_kernel_spmd(nc, in_maps, list(range(NCORES)), trace=trace, **kw)
    # results[c]["out"]: [B, 128, N] -- core c's 128-row shard of y^T per batch
    yT = np.concatenate([res.results[c]["out"] for c in range(NCORES)], axis=1)
    out = np.ascontiguousarray(yT.transpose(0, 2, 1))  # [B, N, C]
    return out, res


def kernel(**inputs) -> np.ndarray:
    out, _ = run_sharded(inputs, trace=False)
    return out


# revision 9
# speedup vs baseline: 1.2676x; 1.2676x over previous
"""Multi-head attention with q/v LoRA on 8 trn2 NeuronCores.

Reference computation (B=2, N=2048, C=1024, H=16, HD=64, R=16):
    qkv = x @ w_qkv + b_qkv                -> split per-head q, k, v
    q  += ((q @ a_q) @ b_q) * 2.0          (per head; same for v)
    out = softmax(q k^T / 8) v             (full N x N scores)
    y   = out @ w_proj + b_proj

Sharding: tensor-parallel over heads for qkv+attention -- each of the 8
cores owns 2 heads (128 of the 1024 qkv columns) for both batches; the
attention output is then resharded over tokens with a 2 MB AllToAll so
each core computes final (not partial) proj rows for its 256 tokens per
batch with the full w_proj.  Per core:
  1. transpose x on the PE (fp32), compute the qkv^T shard in fp32r,
  2. LoRA via block-diagonal [128,32]/[32,128] matrices,
  3. per (batch, head, q-half): scores S^T = k^T' q^T -> exp on ACT ->
     P @ [v | 1] accumulated in PSUM (ones column yields softmax sums),
     normalize with a PE ones-broadcast of the reciprocal sums, DMA the
     normalized O^T slices into the per-batch AllToAll staging buffer,
  4. AllToAll [8, 128, 256] per batch, then proj y^T[:, my 256 tokens]
     = sum_k w_proj[k-chunk]^T @ recv[k-chunk] with bias on every core.
The host stitches the 8 token shards and transposes back to [B, N, C].
"""

import sys

sys.path.insert(0, "/opt/trn_rl_repo")
sys.path.insert(0, "/root/.axon_site")

import numpy as np

import concourse.bass as bass
import concourse.mybir as mybir
import concourse.tile as tile
from concourse.bass_utils import run_bass_kernel_spmd

f32 = mybir.dt.float32
f32r = mybir.dt.float32r
AF = mybir.ActivationFunctionType

B, N, C = 2, 2048, 1024
H, HD, R = 16, 64, 16
LORA_SCALE = 32.0 / R
ATTN_SCALE = HD ** -0.5
NCORES = 8
HPC = H // NCORES          # heads per core = 2
PC = HPC * HD              # partition columns per core = 128
ROWS = B * N               # 4096 tokens
RC = 256                   # row-chunk size for x^T production
TPC = N // NCORES          # tokens per core per batch = 256


def _legalize_waits(nc, max_waits=1):
    """This walrus build accepts at most one sync-wait per instruction;
    Tile attaches several.  Move surplus waits onto same-engine NoOps
    inserted immediately before the instruction (identical semantics)."""
    counter = 0
    for fn in nc.m.functions:
        for bb in fn.blocks:
            insts = bb.instructions
            out = []
            changed = False
            for inst in insts:
                si = inst.sync_info
                if si is not None and si.on_wait and len(si.on_wait) > max_waits:
                    waits = list(si.on_wait)
                    for w in waits[:-max_waits]:
                        counter += 1
                        nop = mybir.InstNoOp(
                            name=f"I-wfix-{counter}",
                            engine=inst.engine,
                            sync_info=mybir.SyncInfo(on_wait=[w], on_update=[]),
                        )
                        nc.register_instruction(nop)
                        out.append(nop)
                    si.on_wait.clear()
                    si.on_wait.extend(waits[-max_waits:])
                    changed = True
                out.append(inst)
            if changed:
                insts[:] = out


def build_nc():
    nc = bass.Bass(num_devices=NCORES)

    x_d = nc.dram_tensor("x", [ROWS, C], f32, kind="ExternalInput")
    wq_d = nc.dram_tensor("wq", [128, 1024], f32, kind="ExternalInput")
    wk_d = nc.dram_tensor("wk", [128, 1024], f32, kind="ExternalInput")
    wv_d = nc.dram_tensor("wv", [128, 1024], f32, kind="ExternalInput")
    bq_d = nc.dram_tensor("bq", [128, 1], f32, kind="ExternalInput")
    bk_d = nc.dram_tensor("bk", [128, 1], f32, kind="ExternalInput")
    bv_d = nc.dram_tensor("bv", [128, 1], f32, kind="ExternalInput")
    a2q_d = nc.dram_tensor("a2q", [128, 2 * R], f32, kind="ExternalInput")
    b2q_d = nc.dram_tensor("b2q", [2 * R, 128], f32, kind="ExternalInput")
    a2v_d = nc.dram_tensor("a2v", [128, 2 * R], f32, kind="ExternalInput")
    b2v_d = nc.dram_tensor("b2v", [2 * R, 128], f32, kind="ExternalInput")
    wp_d = nc.dram_tensor("wp", [128, 8 * 1024], f32, kind="ExternalInput")
    bp_d = nc.dram_tensor("bp", [128, 8], f32, kind="ExternalInput")
    eye128_d = nc.dram_tensor("eye128", [128, 128], f32, kind="ExternalInput")
    eye64x2_d = nc.dram_tensor("eye64x2", [128, 64], f32, kind="ExternalInput")
    out_d = nc.dram_tensor("out", [B, C, TPC], f32, kind="ExternalOutput")

    with nc.allow_low_precision(
        reason="fp32r rounding is intended; PSUM accumulation stays fp32"
    ), tile.TileContext(nc) as tc:
        with (
            tc.tile_pool(name="persist", bufs=1) as persist,
            tc.tile_pool(name="const", bufs=1) as const,
        ):
            qT = persist.tile([128, ROWS], f32r, tag="qT", name="qT")
            kT = persist.tile([128, ROWS], f32r, tag="kT", name="kT")
            vT = persist.tile([128, ROWS], f32r, tag="vT", name="vT")

            # fp32 staging + on-device rounding to fp32r for matmul operands
            def rounded(name, dram, shape):
                stg = const.tile(list(shape), f32, tag="stg", name=f"{name}_stg")
                nc.sync.dma_start(out=stg[:], in_=dram[:])
                t = const.tile(list(shape), f32r, tag=name, name=name)
                nc.vector.tensor_copy(t[:], stg[:])
                return t

            w_t = [
                rounded("wq_t", wq_d, (128, 1024)),
                rounded("wk_t", wk_d, (128, 1024)),
                rounded("wv_t", wv_d, (128, 1024)),
            ]
            a2q_t = rounded("a2q_t", a2q_d, (128, 2 * R))
            b2q_t = rounded("b2q_t", b2q_d, (2 * R, 128))
            a2v_t = rounded("a2v_t", a2v_d, (128, 2 * R))
            b2v_t = rounded("b2v_t", b2v_d, (2 * R, 128))

            # full w_proj, rounded chunk by chunk through the staging slot
            wp_t = const.tile([128, 8 * 1024], f32r, tag="wp_t", name="wp_t")
            for kc in range(8):
                stg = const.tile([128, 1024], f32, tag="stg", name=f"wp_stg{kc}")
                nc.sync.dma_start(out=stg[:], in_=wp_d[:, kc * 1024 : (kc + 1) * 1024])
                nc.vector.tensor_copy(wp_t[:, kc * 1024 : (kc + 1) * 1024], stg[:])

            eye128 = const.tile([128, 128], f32, tag="eye128", name="eye128")
            nc.sync.dma_start(out=eye128[:], in_=eye128_d[:])
            eye64x2_s = const.tile([128, 64], f32, tag="eye64s", name="eye64s")
            nc.sync.dma_start(out=eye64x2_s[:], in_=eye64x2_d[:])
            eye64x2 = const.tile([128, 64], f32r, tag="eye64", name="eye64")
            nc.vector.tensor_copy(eye64x2[:], eye64x2_s[:])

            ones_s = const.tile([128, 64], f32, tag="ones_s", name="ones_s")
            nc.gpsimd.memset(ones_s[:], 1.0)
            ones_row = const.tile([1, 64], f32r, tag="ones_r", name="ones_r")
            nc.vector.tensor_copy(ones_row[:], ones_s[0:1, :])
            ones_col = const.tile([128, 1], f32r, tag="ones_c", name="ones_c")
            nc.vector.tensor_copy(ones_col[:], ones_s[:, 0:1])

            bias_t = []
            for nm, d in (("bq", bq_d), ("bk", bk_d), ("bv", bv_d)):
                bt = const.tile([128, 1], f32, tag=nm, name=f"{nm}_t")
                nc.sync.dma_start(out=bt[:], in_=d[:])
                bias_t.append(bt)
            bp_t = const.tile([128, 8], f32, tag="bp", name="bp_t")
            nc.sync.dma_start(out=bp_t[:], in_=bp_d[:])

            with (
                tc.tile_pool(name="dram", bufs=1, space="DRAM") as dram,
                tc.tile_pool(name="xrow", bufs=4) as xrow_p,
                tc.tile_pool(name="xT", bufs=2) as xT_p,
                tc.tile_pool(name="work", bufs=2) as work_p,
                tc.tile_pool(name="ps", bufs=1, space="PSUM") as ps,
            ):
                qkvT = (qT, kT, vT)
                for b in range(B):
                    boff = b * N
                    # ---- phase A: x^T and qkv^T for this batch ----------
                    for rci in range(N // RC):
                        r0 = boff + rci * RC
                        xrows = []
                        for j in range(RC // 128):
                            xr = xrow_p.tile([128, C], f32, tag="xr", name=f"xr{b}{rci}{j}")
                            nc.sync.dma_start(
                                out=xr[:], in_=x_d[r0 + j * 128 : r0 + (j + 1) * 128, :]
                            )
                            xrows.append(xr)
                        xT_t = xT_p.tile([128, 8 * RC], f32r, tag="xT", name=f"xT{b}{rci}")
                        for ci in range(8):
                            tp = ps.tile([128, RC], f32, tag="s", bufs=2, name=f"tp{rci}{ci}")
                            for j in range(RC // 128):
                                nc.tensor.transpose(
                                    tp[:, j * 128 : (j + 1) * 128],
                                    xrows[j][:, ci * 128 : (ci + 1) * 128],
                                    eye128[:],
                                )
                            if ci % 2 == 0:
                                nc.vector.tensor_copy(
                                    xT_t[:, ci * RC : (ci + 1) * RC], tp[:]
                                )
                            else:
                                nc.scalar.activation(
                                    xT_t[:, ci * RC : (ci + 1) * RC], tp[:], AF.Copy
                                )
                        for m in range(3):
                            acc = ps.tile([128, RC], f32, tag="acc", bufs=2, name=f"ac{rci}{m}")
                            for ci in range(8):
                                nc.tensor.matmul(
                                    acc[:],
                                    w_t[m][:, ci * 128 : (ci + 1) * 128],
                                    xT_t[:, ci * RC : (ci + 1) * RC],
                                    start=(ci == 0),
                                    stop=(ci == 7),
                                )
                            dst = qkvT[m][:, r0 : r0 + RC]
                            if m == 2:
                                nc.vector.tensor_scalar_add(dst, acc[:], bias_t[m][:])
                            else:
                                nc.scalar.activation(
                                    dst, acc[:], AF.Identity, bias=bias_t[m][:]
                                )

                    # ---- LoRA on this batch's q and v -------------------
                    for dstT, a2, b2 in ((qT, a2q_t, b2q_t), (vT, a2v_t, b2v_t)):
                        for ch in range(4):
                            sl = slice(boff + ch * 512, boff + (ch + 1) * 512)
                            t_ps = ps.tile([2 * R, 512], f32, tag="s", bufs=2, name=f"tp{ch}")
                            nc.tensor.matmul(t_ps[:], a2[:], dstT[:, sl], start=True, stop=True)
                            t_sb = work_p.tile([2 * R, 512], f32r, tag="lt", name=f"ts{ch}")
                            nc.vector.tensor_copy(t_sb[:], t_ps[:])
                            d_ps = ps.tile([128, 512], f32, tag="s", bufs=2, name=f"dp{ch}")
                            nc.tensor.matmul(d_ps[:], b2[:], t_sb[:], start=True, stop=True)
                            nc.vector.tensor_add(dstT[:, sl], dstT[:, sl], d_ps[:])

                    # ---- attention: units of (head, q-half) -------------
                    a2a_in = dram.tile([8, 128, TPC], f32, tag=f"ai{b}", name=f"ai{b}")
                    for hl in range(HPC):
                        hs = slice(hl * HD, (hl + 1) * HD)
                        # v_aug for this (batch, head): [v | 1] per k-tile
                        v_aug = work_p.tile([128, 16 * 65], f32r, tag="vaug", name=f"va{b}{hl}")
                        for kt in range(16):
                            ko = boff + kt * 128
                            vtr = ps.tile([128, 64], f32r, tag="s", bufs=2, name=f"vt{kt}")
                            nc.tensor.transpose(
                                vtr[:], vT[hs, ko : ko + 128], eye64x2[hs, :]
                            )
                            nc.vector.tensor_copy(v_aug[:, kt * 65 : kt * 65 + 64], vtr[:])
                            nc.vector.tensor_copy(
                                v_aug[:, kt * 65 + 64 : kt * 65 + 65], ones_col[:]
                            )
                        for qh in range(2):
                            qoff = boff + qh * 1024
                            o_ps = ps.tile([65, 1024], f32, tag="o", bufs=1, name=f"o{b}{hl}{qh}")
                            for kt in range(16):
                                ko = boff + kt * 128
                                s_ps = ps.tile([128, 1024], f32, tag="s", bufs=2, name=f"s{kt}")
                                for qc in range(2):
                                    nc.tensor.matmul(
                                        s_ps[:, qc * 512 : (qc + 1) * 512],
                                        kT[hs, ko : ko + 128],
                                        qT[hs, qoff + qc * 512 : qoff + (qc + 1) * 512],
                                        start=True,
                                        stop=True,
                                    )
                                p_sb = work_p.tile([128, 1024], f32r, tag="p", bufs=3, name=f"p{kt}")
                                nc.scalar.activation(p_sb[:], s_ps[:], AF.Exp, scale=ATTN_SCALE)
                                for qc in range(2):
                                    nc.tensor.matmul(
                                        o_ps[:, qc * 512 : (qc + 1) * 512],
                                        v_aug[:, kt * 65 : kt * 65 + 65],
                                        p_sb[:, qc * 512 : (qc + 1) * 512],
                                        start=(kt == 0),
                                        stop=(kt == 15),
                                    )
                            # normalize rows 0..63 by the sums in row 64
                            r_sb = work_p.tile([1, 1024], f32r, tag="r", name=f"r{b}{hl}{qh}")
                            nc.vector.reciprocal(r_sb[:], o_ps[64:65, :])
                            nst = work_p.tile([64, 1024], f32, tag="nst", bufs=2, name=f"ns{hl}{qh}")
                            for qc in range(2):
                                bc_ps = ps.tile([64, 512], f32, tag="acc", bufs=2, name=f"bc{qc}")
                                nc.tensor.matmul(
                                    bc_ps[:],
                                    ones_row[:],
                                    r_sb[:, qc * 512 : (qc + 1) * 512],
                                    start=True,
                                    stop=True,
                                )
                                bcs = work_p.tile([64, 512], f32, tag="bcs", bufs=2, name=f"bs{qc}")
                                nc.vector.tensor_copy(bcs[:], bc_ps[:])
                                nc.vector.tensor_mul(
                                    nst[:, qc * 512 : (qc + 1) * 512],
                                    o_ps[0:64, qc * 512 : (qc + 1) * 512],
                                    bcs[:],
                                )
                            for tci in range(4):
                                nc.sync.dma_start(
                                    out=a2a_in[qh * 4 + tci, hl * 64 : hl * 64 + 64, :],
                                    in_=nst[:, tci * TPC : (tci + 1) * TPC],
                                )

                    # ---- AllToAll reshard + final proj for my tokens ----
                    a2a_out = dram.tile([8, 128, TPC], f32, tag=f"ao{b}", name=f"ao{b}")
                    nc.gpsimd.collective_compute(
                        "AllToAll",
                        mybir.AluOpType.bypass,
                        replica_groups=[list(range(NCORES))],
                        ins=[a2a_in[:].opt()],
                        outs=[a2a_out[:].opt()],
                    )
                    recv_s = work_p.tile([128, 8 * TPC], f32, tag="rcs", bufs=1, name=f"rs{b}")
                    recv_r = work_p.tile([128, 8 * TPC], f32r, tag="rcr", bufs=1, name=f"rr{b}")
                    for kc in range(8):
                        nc.sync.dma_start(
                            out=recv_s[:, kc * TPC : (kc + 1) * TPC], in_=a2a_out[kc]
                        )
                        nc.vector.tensor_copy(
                            recv_r[:, kc * TPC : (kc + 1) * TPC],
                            recv_s[:, kc * TPC : (kc + 1) * TPC],
                        )
                    for mt in range(8):
                        y_ps = ps.tile([128, TPC], f32, tag="acc", bufs=2, name=f"y{b}{mt}")
                        for kc in range(8):
                            nc.tensor.matmul(
                                y_ps[:],
                                wp_t[:, kc * 1024 + mt * 128 : kc * 1024 + (mt + 1) * 128],
                                recv_r[:, kc * TPC : (kc + 1) * TPC],
                                start=(kc == 0),
                                stop=(kc == 7),
                            )
                        yst = work_p.tile([128, TPC], f32, tag="yst", bufs=3, name=f"ys{b}{mt}")
                        nc.vector.tensor_scalar_add(yst[:], y_ps[:], bp_t[:, mt : mt + 1])
                        nc.sync.dma_start(
                            out=out_d[b, mt * 128 : (mt + 1) * 128, :], in_=yst[:]
                        )

    _legalize_waits(nc)
    return nc


_NC_CACHE = None


def _get_nc():
    global _NC_CACHE
    if _NC_CACHE is None:
        _NC_CACHE = build_nc()
    return _NC_CACHE


def _make_in_maps(inputs):
    x = np.ascontiguousarray(np.asarray(inputs["x"], dtype=np.float32)).reshape(ROWS, C)
    w_qkv = np.asarray(inputs["w_qkv"], dtype=np.float32)
    b_qkv = np.asarray(inputs["b_qkv"], dtype=np.float32)
    a_q = np.asarray(inputs["a_q"], dtype=np.float32)
    b_q = np.asarray(inputs["b_q"], dtype=np.float32)
    a_v = np.asarray(inputs["a_v"], dtype=np.float32)
    b_v = np.asarray(inputs["b_v"], dtype=np.float32)
    w_proj = np.asarray(inputs["w_proj"], dtype=np.float32)
    b_proj = np.asarray(inputs["b_proj"], dtype=np.float32)

    def blkdiag(m):
        z = np.zeros((2 * m.shape[0], 2 * m.shape[1]), dtype=np.float32)
        z[: m.shape[0], : m.shape[1]] = m
        z[m.shape[0] :, m.shape[1] :] = m
        return z

    a2q = blkdiag(a_q)
    b2q = blkdiag(b_q) * LORA_SCALE
    a2v = blkdiag(a_v)
    b2v = blkdiag(b_v) * LORA_SCALE
    eye128 = np.eye(128, dtype=np.float32)
    eye64x2 = np.vstack([np.eye(64, dtype=np.float32)] * 2)

    def warr(w):                              # [1024, n] -> [128, 8*n] chunk-major
        n = w.shape[1]
        return np.ascontiguousarray(
            w.reshape(8, 128, n).transpose(1, 0, 2).reshape(128, 8 * n)
        )

    wp_full = warr(w_proj)                    # [128, 8*1024]
    bp = np.ascontiguousarray(b_proj.reshape(8, 128).T)

    in_maps = []
    for c in range(NCORES):
        in_maps.append(
            {
                "x": x,
                "wq": warr(w_qkv[:, 0 * C + c * PC : 0 * C + (c + 1) * PC]),
                "wk": warr(w_qkv[:, 1 * C + c * PC : 1 * C + (c + 1) * PC]),
                "wv": warr(w_qkv[:, 2 * C + c * PC : 2 * C + (c + 1) * PC]),
                "bq": np.ascontiguousarray(b_qkv[0 * C + c * PC : 0 * C + (c + 1) * PC].reshape(128, 1)),
                "bk": np.ascontiguousarray(b_qkv[1 * C + c * PC : 1 * C + (c + 1) * PC].reshape(128, 1)),
                "bv": np.ascontiguousarray(b_qkv[2 * C + c * PC : 2 * C + (c + 1) * PC].reshape(128, 1)),
                "a2q": a2q,
                "b2q": b2q,
                "a2v": a2v,
                "b2v": b2v,
                "wp": wp_full,
                "bp": bp,
                "eye128": eye128,
                "eye64x2": eye64x2,
            }
        )
    return in_maps


def run_sharded(inputs, trace=False, **kw):
    nc = _get_nc()
    in_maps = _make_in_maps(inputs)
    res = run_bass_kernel_spmd(nc, in_maps, list(range(NCORES)), trace=trace, **kw)
    # results[c]["out"]: [B, C, TPC] -- core c's token shard of final y^T
    yT = np.concatenate([res.results[c]["out"] for c in range(NCORES)], axis=2)
    out = np.ascontiguousarray(yT.transpose(0, 2, 1))  # [B, N, C]
    return out, res


def kernel(**inputs) -> np.ndarray:
    out, _ = run_sharded(inputs, trace=False)
    return out
